# revision 2
# baseline (speedup 1.0000x reference)
"""Trainium2 Bass kernel for nn_GAU_66503273612026 (GAU with diagonal-only attention).

Math (per batch element b, x_b: [T=2048, D=1024]):
    hidden = silu(x_b @ W_hidden + b_hidden)        # [T, 2*TFO]
    v, gate = split(hidden)                          # [T, TFO] each
    z = silu(x_b @ W_qk + b_qk)                      # [T, TFO]
    q = (z*gamma0 + beta0) / sqrt(TFO); k = z*gamma1 + beta1
    sim = q @ k^T                                    # [T, T] (tiny values; no max-sub needed)
    d_i = exp(sim_ii) / sum_j exp(sim_ij)            # diagonal of softmax only
    V = d[:,None] * v * gate
    out_b = (V @ W_out + b_out)^T                    # [NODES, T]
Final output: stack over b -> [B, 1, NODES, T].

Sharding: data-parallel over B: batch element b -> NeuronCore b (8 cores).
Everything on-chip is kept feature-partitioned/token-free ("transposed") so no
runtime transposes are needed; x is pre-transposed on host (data movement only).
Matmuls run as fp32r (fast fp32 mode, 1 PE cycle/row at N=512); q/k/z use bf16
(validated: contributes ~1e-7 relative error because sim values are ~1e-4).
"""

import numpy as np
from contextlib import ExitStack

B, T, D, TFO, NODES = 8, 2048, 1024, 1024, 1024
P = 128
FT = 512            # free-dim tile (one PSUM bank of f32)
NT = T // FT        # 4 token tiles
DC = D // P         # 8 contraction chunks over D
OC = TFO // P       # 8 feature chunks over TFO
NC_ = NODES // P    # 8 output row chunks
IC = T // P         # 16 row chunks for attention stats

_compiled_nc = None


def _build():
    import concourse.bass as bass
    import concourse.tile as tile
    from concourse import bacc, mybir
    from concourse.bass import ts
    from concourse.masks import make_identity

    f32 = mybir.dt.float32
    f32r = mybir.dt.float32r
    bf16 = mybir.dt.bfloat16
    AF = mybir.ActivationFunctionType
    OP = mybir.AluOpType
    AX = mybir.AxisListType

    nc = bacc.Bacc("TRN2", target_bir_lowering=False, debug=False,
                   enable_asserts=False, num_devices=1)

    xT = nc.dram_tensor("xT", [D, T], f32r, kind="ExternalInput").ap()
    wqk = nc.dram_tensor("wqk", [D, TFO], f32r, kind="ExternalInput").ap()
    wh = nc.dram_tensor("wh", [D, 2 * TFO], f32r, kind="ExternalInput").ap()
    wo = nc.dram_tensor("wo", [TFO, NODES], f32r, kind="ExternalInput").ap()
    # per-chunk column layouts [P, n_chunks]: column c holds elements c*128..c*128+127
    bqk = nc.dram_tensor("bqk", [P, OC], f32, kind="ExternalInput").ap()
    bv = nc.dram_tensor("bv", [P, OC], f32, kind="ExternalInput").ap()
    bg = nc.dram_tensor("bg", [P, OC], f32, kind="ExternalInput").ap()
    bo = nc.dram_tensor("bo", [P, NC_], f32, kind="ExternalInput").ap()
    g0 = nc.dram_tensor("g0", [P, OC], f32, kind="ExternalInput").ap()   # gamma0/32
    be0 = nc.dram_tensor("be0", [P, OC], f32, kind="ExternalInput").ap()  # beta0/32
    g1 = nc.dram_tensor("g1", [P, OC], f32, kind="ExternalInput").ap()
    be1 = nc.dram_tensor("be1", [P, OC], f32, kind="ExternalInput").ap()
    outT = nc.dram_tensor("outT", [NODES, T], f32, kind="ExternalOutput").ap()

    with tile.TileContext(nc) as tc, ExitStack() as ctx:
        persist = ctx.enter_context(tc.tile_pool(name="persist", bufs=1))
        dramp = ctx.enter_context(tc.tile_pool(name="dram", bufs=1, space="DRAM"))

        # constants
        bqk_sb = persist.tile([P, OC], f32, tag="bqk")
        bv_sb = persist.tile([P, OC], f32, tag="bv")
        bg_sb = persist.tile([P, OC], f32, tag="bg")
        bo_sb = persist.tile([P, NC_], f32, tag="bo")
        g0_sb = persist.tile([P, OC], f32, tag="g0")
        be0_sb = persist.tile([P, OC], f32, tag="be0")
        g1_sb = persist.tile([P, OC], f32, tag="g1")
        be1_sb = persist.tile([P, OC], f32, tag="be1")
        for sb, dr in ((bqk_sb, bqk), (bv_sb, bv), (bg_sb, bg), (bo_sb, bo),
                       (g0_sb, g0), (be0_sb, be0), (g1_sb, g1), (be1_sb, be1)):
            nc.sync.dma_start(out=sb, in_=dr)
        ident = persist.tile([P, P], f32, tag="ident")
        make_identity(nc, ident[:])

        # x^T resident: [p, dc, t] where d = dc*128+p
        x_sb = persist.tile([P, DC, T], f32r, tag="x")
        for dc in range(DC):
            nc.sync.dma_start(out=x_sb[:, dc, :], in_=xT[ts(dc, P), :])

        dbcast = persist.tile([P, T], f32, tag="dbcast")   # diag row broadcast
        diag_dr = dramp.tile([T, 1], f32, tag="diag")      # DRAM scratch

        # ---------------- Phase A: z^T = silu(x @ W_qk + b_qk), bf16
        # ---------------- Phase B: attention diag stats
        with ExitStack() as ab:
            abp = ab.enter_context(tc.tile_pool(name="ab", bufs=1))
            wqkp = ab.enter_context(tc.tile_pool(name="wqkp", bufs=2))
            qp = ab.enter_context(tc.tile_pool(name="qp", bufs=2))
            ep = ab.enter_context(tc.tile_pool(name="ep", bufs=3))
            statp = ab.enter_context(tc.tile_pool(name="statp", bufs=4))
            psA = ab.enter_context(tc.tile_pool(name="psA", bufs=3, space="PSUM"))
            psB = ab.enter_context(tc.tile_pool(name="psB", bufs=2, space="PSUM"))

            z_sb = abp.tile([P, OC, T], bf16, tag="z")
            k_sb = abp.tile([P, OC, T], bf16, tag="k")
            wqk_r = wqk.rearrange("(dc p) e -> p dc e", p=P)
            for oc in range(OC):
                w = wqkp.tile([P, DC, P], f32r, tag="wqk")
                nc.sync.dma_start(out=w, in_=wqk_r[:, :, ts(oc, P)])
                for t in range(NT):
                    ps = psA.tile([P, FT], f32, tag="zps")
                    for dc in range(DC):
                        nc.tensor.matmul(ps[:], lhsT=w[:, dc, :],
                                         rhs=x_sb[:, dc, ts(t, FT)],
                                         start=(dc == 0), stop=(dc == DC - 1))
                    nc.scalar.activation(out=z_sb[:, oc, ts(t, FT)], in_=ps[:],
                                         func=AF.Silu, bias=bqk_sb[:, oc:oc + 1],
                                         scale=1.0)
                # k chunk for this oc (overlaps with next oc's matmuls)
                nc.vector.tensor_scalar(out=k_sb[:, oc, :], in0=z_sb[:, oc, :],
                                        scalar1=g1_sb[:, oc:oc + 1],
                                        scalar2=be1_sb[:, oc:oc + 1],
                                        op0=OP.mult, op1=OP.add)

            for ic in range(IC):
                q_t = qp.tile([P, OC, P], bf16, tag="q")
                for oc in range(OC):
                    nc.vector.tensor_scalar(out=q_t[:, oc, :],
                                            in0=z_sb[:, oc, ts(ic, P)],
                                            scalar1=g0_sb[:, oc:oc + 1],
                                            scalar2=be0_sb[:, oc:oc + 1],
                                            op0=OP.mult, op1=OP.add)
                rs4 = statp.tile([P, NT], f32, tag="rs4")
                dnum = statp.tile([P, 1], f32, tag="dnum")
                for jt in range(NT):
                    ps = psB.tile([P, FT], f32, tag="simps")
                    for oc in range(OC):
                        nc.tensor.matmul(ps[:], lhsT=q_t[:, oc, :],
                                         rhs=k_sb[:, oc, ts(jt, FT)],
                                         start=(oc == 0), stop=(oc == OC - 1))
                    # exp + fused row-sum (no max subtraction: |sim| < 1e-3)
                    et = ep.tile([P, FT], f32, tag="exp")
                    nc.scalar.activation(out=et[:], in_=ps[:], func=AF.Exp,
                                         accum_out=rs4[:, jt:jt + 1])
                    if jt == ic // NT:
                        off = (ic % NT) * P
                        tmp = ep.tile([P, P], f32, tag="dtmp")
                        dsim = statp.tile([P, 1], f32, tag="dsim")
                        nc.vector.scalar_tensor_tensor(
                            out=tmp[:], in0=ps[:, off:off + P], scalar=1.0,
                            in1=ident[:], op0=OP.mult, op1=OP.mult,
                            accum_out=dsim[:])
                        nc.scalar.activation(out=dnum[:], in_=dsim[:], func=AF.Exp)
                s = statp.tile([P, 1], f32, tag="s")
                nc.vector.reduce_sum(out=s[:], in_=rs4[:], axis=AX.X)
                sinv = statp.tile([P, 1], f32, tag="sinv")
                nc.vector.reciprocal(sinv[:], s[:])
                dcol = statp.tile([P, 1], f32, tag="dcol")
                nc.vector.tensor_tensor(out=dcol[:], in0=dnum[:], in1=sinv[:],
                                        op=OP.mult)
                nc.sync.dma_start(out=diag_dr[ts(ic, P), :], in_=dcol[:])

            # broadcast diag row to all partitions: [P, T]
            scr_ap = diag_dr[:]
            bc_ap = bass.AP(tensor=scr_ap.tensor, offset=scr_ap.offset,
                            ap=[[0, P], [1, T]])
            nc.gpsimd.dma_start(out=dbcast[:], in_=bc_ap)

        # ---------------- Phase C: V^T = silu(xWv+bv)*silu(xWg+bg)*diag, out = (W_out^T @ V^T) + b_out
        with ExitStack() as cc:
            cp = cc.enter_context(tc.tile_pool(name="cp", bufs=1))
            whp = cc.enter_context(tc.tile_pool(name="whp", bufs=2))
            stg = cc.enter_context(tc.tile_pool(name="stg", bufs=3))
            psC = cc.enter_context(tc.tile_pool(name="psC", bufs=2, space="PSUM"))

            V_sb = cp.tile([P, OC, T], f32r, tag="V")
            wh_r = wh.rearrange("(dc p) e -> p dc e", p=P)
            for oc in range(OC):
                wv = whp.tile([P, DC, P], f32r, tag="wv")
                nc.sync.dma_start(out=wv, in_=wh_r[:, :, ts(oc, P)])
                wg = whp.tile([P, DC, P], f32r, tag="wg")
                nc.sync.dma_start(out=wg, in_=wh_r[:, :, ts(OC + oc, P)])
                for t in range(NT):
                    vps = psC.tile([P, FT], f32, tag="vps")
                    for dc in range(DC):
                        nc.tensor.matmul(vps[:], lhsT=wv[:, dc, :],
                                         rhs=x_sb[:, dc, ts(t, FT)],
                                         start=(dc == 0), stop=(dc == DC - 1))
                    gps = psC.tile([P, FT], f32, tag="gps")
                    for dc in range(DC):
                        nc.tensor.matmul(gps[:], lhsT=wg[:, dc, :],
                                         rhs=x_sb[:, dc, ts(t, FT)],
                                         start=(dc == 0), stop=(dc == DC - 1))
                    sv = stg.tile([P, FT], f32, tag="sv")
                    nc.scalar.activation(out=sv[:], in_=vps[:], func=AF.Silu,
                                         bias=bv_sb[:, oc:oc + 1])
                    sg = stg.tile([P, FT], f32, tag="sg")
                    nc.scalar.activation(out=sg[:], in_=gps[:], func=AF.Silu,
                                         bias=bg_sb[:, oc:oc + 1])
                    pv = stg.tile([P, FT], f32, tag="pv")
                    nc.vector.tensor_tensor(out=pv[:], in0=sv[:], in1=sg[:],
                                            op=OP.mult)
                    nc.vector.tensor_tensor(out=V_sb[:, oc, ts(t, FT)], in0=pv[:],
                                            in1=dbcast[:, ts(t, FT)], op=OP.mult)

            wo_r = wo.rearrange("(oc p) n -> p oc n", p=P)
            for ncb in range(NC_):
                wot = whp.tile([P, OC, P], f32r, tag="wo")
                nc.sync.dma_start(out=wot, in_=wo_r[:, :, ts(ncb, P)])
                for t in range(NT):
                    ops = psC.tile([P, FT], f32, tag="ops")
                    for oc in range(OC):
                        nc.tensor.matmul(ops[:], lhsT=wot[:, oc, :],
                                         rhs=V_sb[:, oc, ts(t, FT)],
                                         start=(oc == 0), stop=(oc == OC - 1))
                    ost = stg.tile([P, FT], f32, tag="ost")
                    nc.scalar.activation(out=ost[:], in_=ops[:], func=AF.Identity,
                                         bias=bo_sb[:, ncb:ncb + 1])
                    nc.sync.dma_start(out=outT[ts(ncb, P), ts(t, FT)], in_=ost[:])

    nc.compile()
    return nc


def _get_nc():
    global _compiled_nc
    if _compiled_nc is None:
        _compiled_nc = _build()
    return _compiled_nc


def _cols(v, n):
    return np.ascontiguousarray(np.asarray(v, dtype=np.float32).reshape(n, P).T)


def kernel(x, W_hidden, b_hidden, W_qk, b_qk, gamma, beta, W_out, b_out):
    from concourse.bass_utils import run_bass_kernel_spmd

    nc = _get_nc()
    x = np.asarray(x, dtype=np.float32)
    gamma = np.asarray(gamma, dtype=np.float32)
    beta = np.asarray(beta, dtype=np.float32)
    shared = {
        "wqk": np.asarray(W_qk, dtype=np.float32),
        "wh": np.asarray(W_hidden, dtype=np.float32),
        "wo": np.asarray(W_out, dtype=np.float32),
        "bqk": _cols(b_qk, OC),
        "bv": _cols(np.asarray(b_hidden, dtype=np.float32)[:TFO], OC),
        "bg": _cols(np.asarray(b_hidden, dtype=np.float32)[TFO:], OC),
        "bo": _cols(b_out, NC_),
        "g0": _cols(gamma[0] / 32.0, OC),
        "be0": _cols(beta[0] / 32.0, OC),
        "g1": _cols(gamma[1], OC),
        "be1": _cols(beta[1], OC),
    }
    in_maps = [dict(shared, xT=np.ascontiguousarray(x[b].T)) for b in range(B)]
    res = run_bass_kernel_spmd(nc, in_maps, list(range(B)))
    out = np.stack([res.results[b]["outT"] for b in range(B)])[:, None]
    return out
